# revision 11
# baseline (speedup 1.0000x reference)
# Trainium2 Bass kernel for nn_FCM_series_1 (gnn_message_passing).
#
# Math (derived from the reference):
#   aggregate(X, WW)[l,b,j] = tanh(-sum_i X[l,b,i] * WW[i,j])
#   T_A  = aggregate(A, WW)                     (12 lags x B rows)
#   U[t] = aggregate(train_init[:,:,t,1], WW)   (13 unique rows per batch;
#          A_N_OLD[la] = U[la], A_0_NEW[la] = U[la+1])
#   out[b,la,j] = P[la,j]*T_A[la,b,j] + Q[la,j]*U[la+1,b,j] + R[la,j]*U[la,b,j]
# with host-computable coefficients
#   P[la,j] = 2 * lambd[la, j%200] / belta[la] * 3**fract[la]
#   Q[la,j] = 3 * lambd[la, j%200] * l[la, j%200] / belta[la]
#   R[la,j] = Q[la,j] * Gamma(a+1)/(6*Gamma(a-2))
#   belta[la] = sum_{k=0..3} Gamma(a+1)/(Gamma(k+1)*Gamma(a-k+1))
#
# Sharding over 8 cores: batch split x2 (16 each), output node dim j split x4
# (300 each). Per core one matmul chain: lhsT=W k-tiles, rhs=X^T k-tiles,
# PSUM-accumulated over 10 k-tiles of 120, all operands bf16 (W negated on
# the host so psum = -X@W; bf16 streaming is ~2e-3 rel err, far under the
# 2e-2 gate).
#
# Schedule notes (why it looks like this):
# - Everything before the first real matmul is overhead; the PE clock (HAM)
#   additionally needs ~6us of CONTINUOUS busy to reach full speed (cold
#   cadence ~2x slower, and any idle gap resets the ramp). So the GpSimd
#   engine memsets the warmup scratch first thing and the PE runs throwaway
#   matmuls back-to-back from ~6.2us until the real stream takes over.
# - bf16-direct streaming (no int8+dequant) keeps the convert stage OFF the
#   critical path entirely: the first matmul only waits for the Wk0-1/Xk0-1
#   DMAs. ACT only does the 3 tanhs, DVE only the coefficient replicate and
#   the 5 merged combines.
# - Each dma_start costs ~7ns/descriptor-row of engine issue time and the
#   queue transfers follow descriptor writes, so few, large-row transfers
#   win. Rows here are 1200-3200B.
# - The Scalar engine's auto-emitted tanh-table load (~1.3us) gates its
#   first issue, so the k0-1 chunks ride the Sync queue.
# - Matmul order: k0-5 across all jt, then per-jt k6-9, so jt0's tanh and
#   the merged combines start as early as possible after the k6-9 chunks
#   land. Epilogue = 3 tanh (ACT, psum->bf16), 5 merged DVE ops via 3D APs
#   spanning all 3 j-subtiles, one flat output DMA ([JS, NJ*CA] bf16, host
#   untangles).

import math

import numpy as np

LAG = 13
B = 32
N = 1200
H = 1.0 / 3.0

PB = 2          # batch shards
PJ = 4          # j shards
BL = B // PB    # 16 batches per core
JL = N // PJ    # 300 output nodes per core
NL = LAG - 1    # 12
CA = NL * BL    # 192 cols: T_A block, col = la*BL + b
CU = LAG * BL   # 208 cols: U block,  col = CA + t*BL + b
C = CA + CU     # 400 matmul moving cols
KT = 120        # contraction tile
NK = N // KT    # 10
JS = 100        # j subtile (psum partition dim)
NJ = JL // JS   # 3 j subtiles per core
N_WARMUP = 9

_cached = None


def _gamma(x):
    return math.gamma(x)


def _build_nc():
    import concourse.bacc as bacc
    import concourse.mybir as mybir
    from concourse.tile import TileContext

    bf16 = mybir.dt.bfloat16
    # enable_partition_id=False drops the per-engine partition-id TENSOR_LOAD
    # from the preamble (~1.2us on the critical path; this kernel never reads
    # the partition id).
    nc = bacc.Bacc(None, target_bir_lowering=False, enable_partition_id=False)

    # partition-major repacked inputs (see kernel() for layouts)
    xt = nc.dram_tensor("xt", [KT, NK * C], bf16, kind="ExternalInput")
    wc = nc.dram_tensor("wc", [KT, NK * JL], bf16, kind="ExternalInput")
    coef = nc.dram_tensor("coef", [JS, 3 * NJ * NL], bf16, kind="ExternalInput")
    out = nc.dram_tensor("out", [JS, NJ * CA], bf16, kind="ExternalOutput")

    with TileContext(nc) as tc:
        with (
            tc.tile_pool(name="sb", bufs=1) as pool,
            tc.tile_pool(name="ps", bufs=1, space="PSUM") as pspool,
        ):
            # Each chunk gets its OWN tile: slicing one big tile lumps the
            # DMA-write dependencies, making the first matmul wait on later
            # chunks. Per-queue sustained throughput is ~215GB/s (aggregate
            # ~340 when both HWDGE queues pull), and each transfer carries
            # ~2.2us of fixed latency (SEQ issue + DGE start + 900ns
            # completion-semaphore propagation), so: the k0 pair rides the
            # GpSimd SWDGE queue (that engine frees earliest, ~1us before
            # Sync can issue), Sync carries k1 + the X bulk, Scalar (whose
            # first issue waits ~1.3us on its tanh-table load) carries the W
            # bulk, sized so both queues drain together.
            wg = {}   # k0 -> W chunk tile
            xg = {}   # k0 -> X chunk tile

            def loadw(eng, k0, nk):
                g = pool.tile([KT, nk * JL], bf16, tag=f"wg{k0}",
                              name=f"wg{k0}")
                eng.dma_start(out=g[:], in_=wc[:, k0 * JL:(k0 + nk) * JL])
                wg[k0] = g

            def loadx(eng, k0, nk):
                g = pool.tile([KT, nk * C], bf16, tag=f"xg{k0}",
                              name=f"xg{k0}")
                eng.dma_start(out=g[:], in_=xt[:, k0 * C:(k0 + nk) * C])
                xg[k0] = g

            scratch = pool.tile([KT, C], bf16, tag="scr")
            nc.gpsimd.memset(scratch[:], 0)
            loadw(nc.gpsimd, 0, 1)
            loadx(nc.gpsimd, 0, 1)
            coef_all = pool.tile([JS, 3 * NJ * NL], bf16, tag="coef")
            nc.gpsimd.dma_start(out=coef_all[:], in_=coef[:, :])

            loadw(nc.sync, 1, 1)
            loadx(nc.sync, 1, 1)
            loadx(nc.sync, 2, 4)
            loadx(nc.sync, 6, 2)
            loadx(nc.sync, 8, 2)
            loadw(nc.scalar, 2, 4)
            loadw(nc.scalar, 6, 2)
            loadw(nc.scalar, 8, 2)
            wmap = {k: (wg[0], 0) if k < 1 else (
                (wg[1], 1) if k < 2 else (
                    (wg[2], 2) if k < 6 else (
                        (wg[6], 6) if k < 8 else (wg[8], 8))))
                for k in range(NK)}
            xmap = {k: (xg[0], 0) if k < 1 else (
                (xg[1], 1) if k < 2 else (
                    (xg[2], 2) if k < 6 else (
                        (xg[6], 6) if k < 8 else (xg[8], 8))))
                for k in range(NK)}

            # PE warmups: start as soon as scratch is set, bridge seamlessly
            # into the real matmul stream to keep the HAM ramp alive (cold
            # cadence is ~2x warm; an idle gap triggers a fixed ~3.4us
            # half-speed window).
            psw = pspool.tile([JS, C], mybir.dt.float32, tag="psw", name="psw")
            for _ in range(N_WARMUP):
                nc.tensor.matmul(psw[:], scratch[:, 0:JS], scratch[:],
                                 start=True, stop=True)

            def filler():
                # cheap 100-col matmul: keeps the PE busy across a DMA-gate
                # stall for ~85ns warm instead of idling (which would cost a
                # 3.4us HAM re-throttle window)
                nc.tensor.matmul(psw[:, 0:JS], scratch[:, 0:JS],
                                 scratch[:, 0:JS], start=True, stop=True)

            def w_slice(jt, k):
                g, k0 = wmap[k]
                c0 = (k - k0) * JL + jt * JS
                return g[:, c0:c0 + JS]

            def x_slice(k):
                g, k0 = xmap[k]
                return g[:, (k - k0) * C:(k - k0 + 1) * C]

            # replicate [JS,12] coefficient vectors to [JS,192] in one 4D-AP
            # copy (coef lands early on the gpsimd queue) so the combines run
            # on flat APs.
            crep = pool.tile([JS, 3 * NJ * CA], bf16, tag="crep")
            src = coef_all[:].rearrange("p (g l) -> p g l", g=3 * NJ)
            dst = crep[:].rearrange("p (g l b) -> p g l b", g=3 * NJ, l=NL)
            nc.vector.tensor_copy(dst, src.broadcast_to([JS, 3 * NJ, NL, BL]))

            ps = [pspool.tile([JS, C], mybir.dt.float32, tag=f"ps{jt}",
                              name=f"ps{jt}")
                  for jt in range(NJ)]
            mm_order = [(jt, k) for k in range(6) for jt in range(NJ)]
            mm_order += [(jt, k) for jt in range(NJ) for k in range(6, NK)]
            for i, (jt, k) in enumerate(mm_order):
                if (jt, k) == (0, 2):
                    for _ in range(4):
                        filler()
                if (jt, k) == (0, 6):
                    for _ in range(2):
                        filler()
                nc.tensor.matmul(
                    ps[jt][:], w_slice(jt, k), x_slice(k),
                    start=(k == 0), stop=(k == NK - 1),
                )

            # epilogue: per-jt tanh on ACT, merged 3-jt combines on DVE,
            # one flat output DMA.
            t_all = pool.tile([JS, NJ * C], bf16, tag="t")
            res = pool.tile([JS, NJ * CA], bf16, tag="res")
            tmp = pool.tile([JS, NJ * CA], bf16, tag="tmp")
            tmp2 = pool.tile([JS, NJ * CA], bf16, tag="tmp2")
            for jt in range(NJ):
                nc.scalar.activation(
                    out=t_all[:, jt * C:(jt + 1) * C], in_=ps[jt][:],
                    func=mybir.ActivationFunctionType.Tanh,
                )
            t3 = t_all[:].rearrange("p (j c) -> p j c", j=NJ)

            def cre(i):
                return crep[:, i * NJ * CA:(i + 1) * NJ * CA].rearrange(
                    "p (j c) -> p j c", j=NJ)

            res3 = res[:].rearrange("p (j c) -> p j c", j=NJ)
            tmp3 = tmp[:].rearrange("p (j c) -> p j c", j=NJ)
            tmp23 = tmp2[:].rearrange("p (j c) -> p j c", j=NJ)
            ve = nc.vector
            ve.tensor_mul(res3, cre(0), t3[:, :, 0:CA])
            ve.tensor_mul(tmp3, cre(1), t3[:, :, CA + BL:CA + CU])
            ve.tensor_mul(tmp23, cre(2), t3[:, :, CA:CA + CA])
            ve.tensor_add(res[:], res[:], tmp[:])
            ve.tensor_add(res[:], res[:], tmp2[:])
            nc.sync.dma_start(out=out[:, :], in_=res[:])

    return nc


def _get_nc():
    global _cached
    if _cached is None:
        _cached = _build_nc()
        _cached.finalize()   # Bacc: runs reg alloc + codegen passes
    return _cached


def _host_coefs(alpha, fract, lambd, l):
    # All [12,...] fp32; compute in float64, cast at the end.
    a = alpha[:, 0].astype(np.float64)          # [12]
    f = fract[:, 0].astype(np.float64)          # [12]
    lam = lambd[:, 0, :, 0].astype(np.float64)  # [12, 200]
    ll = l[:, 0, :, 0].astype(np.float64)       # [12, 200]

    belta = np.zeros(NL)
    for la in range(NL):
        g_a1 = _gamma(a[la] + 1.0)
        belta[la] = sum(
            g_a1 / (_gamma(kk + 1.0) * _gamma(a[la] - kk + 1.0)) for kk in range(4)
        )
    cN = np.array([_gamma(a[la] + 1.0) / (6.0 * _gamma(a[la] - 2.0))
                   for la in range(NL)])

    # tile lambda/l from 200 -> 1200 (index n % 200)
    lam_t = np.tile(lam, (1, 6))                # [12, 1200]
    ll_t = np.tile(ll, (1, 6))                  # [12, 1200]

    inv_hf = (1.0 / H) ** f                     # 3**fract
    P = 2.0 * lam_t / belta[:, None] * inv_hf[:, None]
    Q = lam_t * ll_t / belta[:, None] / H
    R = Q * cN[:, None]
    return P, Q, R


def kernel(A, WW, train_init, alpha, fract, lambd, l, A_y_list):
    import ml_dtypes
    from concourse.bass_utils import run_bass_kernel_spmd

    bf16 = ml_dtypes.bfloat16

    A = np.asarray(A, dtype=np.float32)
    WW = np.asarray(WW, dtype=np.float32)
    train_init = np.asarray(train_init, dtype=np.float32)

    P, Q, R = _host_coefs(
        np.asarray(alpha, np.float32), np.asarray(fract, np.float32),
        np.asarray(lambd, np.float32), np.asarray(l, np.float32))

    Wneg = (-WW[:, :, 0]).astype(bf16)          # [1200, 1200]

    xts, wcs, coefs = {}, {}, {}
    for beta in range(PB):
        bsl = slice(beta * BL, (beta + 1) * BL)
        xa = A[:, bsl, :, 0].astype(bf16).transpose(2, 0, 1).reshape(N, CA)
        xu = train_init[bsl, :, :, 1].astype(bf16).transpose(1, 2, 0).reshape(
            N, CU)
        XT = np.concatenate([xa, xu], axis=1)                   # [1200, 400]
        # partition-major: [KT, NK*C], col = k*C + c
        xts[beta] = np.ascontiguousarray(
            XT.reshape(NK, KT, C).transpose(1, 0, 2).reshape(KT, NK * C))
    for g in range(PJ):
        gsl = slice(g * JL, (g + 1) * JL)
        # partition-major, k-major: col = k*JL + j
        wcs[g] = np.ascontiguousarray(
            Wneg[:, gsl].reshape(NK, KT, JL).transpose(1, 0, 2).reshape(
                KT, NK * JL))
        # coef [JS, 108]: col = kind*36 + jt*12 + la
        kinds = [M[:, gsl].reshape(NL, NJ, JS).transpose(2, 1, 0)
                 for M in (P, Q, R)]                            # [100, 3, 12]
        coefs[g] = np.ascontiguousarray(
            np.stack(kinds, axis=1).reshape(JS, 3 * NJ * NL).astype(bf16))

    in_maps = []
    for core in range(PB * PJ):
        beta, g = divmod(core, PJ)
        in_maps.append({"xt": xts[beta], "wc": wcs[g], "coef": coefs[g]})

    nc = _get_nc()
    res = run_bass_kernel_spmd(nc, in_maps, core_ids=list(range(PB * PJ)))
    kernel.last_results = res

    full = np.empty((B, NL, N), dtype=np.float32)
    for core in range(PB * PJ):
        beta, g = divmod(core, PJ)
        o = res.results[core]["out"]            # [JS, NJ*CA] bf16
        o = np.asarray(o).astype(np.float32).reshape(JS, NJ, NL, BL)
        # out[p, jt, la, b] -> full[b, la, g*JL + jt*JS + p]
        full[beta * BL:(beta + 1) * BL, :, g * JL:(g + 1) * JL] = (
            o.transpose(3, 2, 1, 0).reshape(BL, NL, JL))
    return full.reshape(B, NL, N, 1)


# revision 13
# speedup vs baseline: 1.1178x; 1.1178x over previous
# Trainium2 Bass kernel for nn_FCM_series_1 (gnn_message_passing).
#
# Math (derived from the reference):
#   aggregate(X, WW)[l,b,j] = tanh(-sum_i X[l,b,i] * WW[i,j])
#   T_A  = aggregate(A, WW)                     (12 lags x B rows)
#   U[t] = aggregate(train_init[:,:,t,1], WW)   (13 unique rows per batch;
#          A_N_OLD[la] = U[la], A_0_NEW[la] = U[la+1])
#   out[b,la,j] = P[la,j]*T_A[la,b,j] + Q[la,j]*U[la+1,b,j] + R[la,j]*U[la,b,j]
# with host-computable coefficients
#   P[la,j] = 2 * lambd[la, j%200] / belta[la] * 3**fract[la]
#   Q[la,j] = 3 * lambd[la, j%200] * l[la, j%200] / belta[la]
#   R[la,j] = Q[la,j] * Gamma(a+1)/(6*Gamma(a-2))
#   belta[la] = sum_{k=0..3} Gamma(a+1)/(Gamma(k+1)*Gamma(a-k+1))
#
# Sharding over 8 cores: batch split x2 (16 each), output node dim j split x4
# (300 each). Per core one matmul chain: lhsT=W k-tiles, rhs=X^T k-tiles,
# PSUM-accumulated over 10 k-tiles of 120, all operands bf16 (W negated on
# the host so psum = -X@W; bf16 streaming is ~2e-3 rel err, far under the
# 2e-2 gate).
#
# Schedule notes (why it looks like this):
# - Everything before the first real matmul is overhead; the PE clock (HAM)
#   additionally needs ~6us of CONTINUOUS busy to reach full speed (cold
#   cadence ~2x slower, and any idle gap resets the ramp). So the GpSimd
#   engine memsets the warmup scratch first thing and the PE runs throwaway
#   matmuls back-to-back from ~6.2us until the real stream takes over.
# - bf16-direct streaming (no int8+dequant) keeps the convert stage OFF the
#   critical path entirely: the first matmul only waits for the Wk0-1/Xk0-1
#   DMAs. ACT only does the 3 tanhs, DVE only the coefficient replicate and
#   the 5 merged combines.
# - Each dma_start costs ~7ns/descriptor-row of engine issue time and the
#   queue transfers follow descriptor writes, so few, large-row transfers
#   win. Rows here are 1200-3200B.
# - The Scalar engine's auto-emitted tanh-table load (~1.3us) gates its
#   first issue, so the k0-1 chunks ride the Sync queue.
# - Matmul order: k0-5 across all jt, then per-jt k6-9, so jt0's tanh and
#   the merged combines start as early as possible after the k6-9 chunks
#   land. Epilogue = 3 tanh (ACT, psum->bf16), 5 merged DVE ops via 3D APs
#   spanning all 3 j-subtiles, one flat output DMA ([JS, NJ*CA] bf16, host
#   untangles).

import math

import numpy as np

LAG = 13
B = 32
N = 1200
H = 1.0 / 3.0

PB = 2          # batch shards
PJ = 4          # j shards
BL = B // PB    # 16 batches per core
JL = N // PJ    # 300 output nodes per core
NL = LAG - 1    # 12
CA = NL * BL    # 192 cols: T_A block, col = la*BL + b
CU = LAG * BL   # 208 cols: U block,  col = CA + t*BL + b
C = CA + CU     # 400 matmul moving cols
KT = 120        # contraction tile
NK = N // KT    # 10
JS = 100        # j subtile (psum partition dim)
NJ = JL // JS   # 3 j subtiles per core
N_WARMUP = 9

_cached = None


def _gamma(x):
    return math.gamma(x)


def _build_nc():
    import concourse.bacc as bacc
    import concourse.mybir as mybir
    from concourse.tile import TileContext

    bf16 = mybir.dt.bfloat16
    # enable_partition_id=False drops the per-engine partition-id TENSOR_LOAD
    # from the preamble (~1.2us on the critical path; this kernel never reads
    # the partition id).
    nc = bacc.Bacc(None, target_bir_lowering=False, enable_partition_id=False)

    # partition-major repacked inputs (see kernel() for layouts)
    xt = nc.dram_tensor("xt", [KT, NK * C], bf16, kind="ExternalInput")
    wc = nc.dram_tensor("wc", [KT, NK * JL], bf16, kind="ExternalInput")
    coef = nc.dram_tensor("coef", [JS, 3 * NJ * NL], bf16, kind="ExternalInput")
    out = nc.dram_tensor("out", [JS, NJ * CA], bf16, kind="ExternalOutput")

    with TileContext(nc) as tc:
        with (
            tc.tile_pool(name="sb", bufs=1) as pool,
            tc.tile_pool(name="ps", bufs=1, space="PSUM") as pspool,
        ):
            # Each chunk gets its OWN tile: slicing one big tile lumps the
            # DMA-write dependencies, making the first matmul wait on later
            # chunks. Per-queue sustained throughput is ~215GB/s (aggregate
            # ~340 when both HWDGE queues pull), and each transfer carries
            # ~2.2us of fixed latency (SEQ issue + DGE start + 900ns
            # completion-semaphore propagation), so: the k0 pair rides the
            # GpSimd SWDGE queue (that engine frees earliest, ~1us before
            # Sync can issue), Sync carries k1 + the X bulk, Scalar (whose
            # first issue waits ~1.3us on its tanh-table load) carries the W
            # bulk, sized so both queues drain together.
            wg = {}   # k0 -> W chunk tile
            xg = {}   # k0 -> X chunk tile

            def loadw(eng, k0, nk):
                g = pool.tile([KT, nk * JL], bf16, tag=f"wg{k0}",
                              name=f"wg{k0}")
                eng.dma_start(out=g[:], in_=wc[:, k0 * JL:(k0 + nk) * JL])
                wg[k0] = g

            def loadx(eng, k0, nk):
                g = pool.tile([KT, nk * C], bf16, tag=f"xg{k0}",
                              name=f"xg{k0}")
                eng.dma_start(out=g[:], in_=xt[:, k0 * C:(k0 + nk) * C])
                xg[k0] = g

            scratch = pool.tile([KT, C], bf16, tag="scr")
            nc.gpsimd.memset(scratch[:], 0)
            coef_all = pool.tile([JS, 3 * NJ * NL], bf16, tag="coef")
            nc.gpsimd.dma_start(out=coef_all[:], in_=coef[:, :])

            loadw(nc.sync, 0, 1)
            loadx(nc.sync, 0, 1)
            loadw(nc.sync, 1, 1)
            loadx(nc.sync, 1, 1)
            loadw(nc.sync, 2, 4)
            loadw(nc.sync, 6, 4)
            loadx(nc.scalar, 2, 4)
            loadx(nc.scalar, 6, 2)
            loadx(nc.scalar, 8, 2)
            wmap = {k: (wg[0], 0) if k < 1 else (
                (wg[1], 1) if k < 2 else (
                    (wg[2], 2) if k < 6 else (wg[6], 6)))
                for k in range(NK)}
            xmap = {k: (xg[0], 0) if k < 1 else (
                (xg[1], 1) if k < 2 else (
                    (xg[2], 2) if k < 6 else (
                        (xg[6], 6) if k < 8 else (xg[8], 8))))
                for k in range(NK)}

            # PE warmups: start as soon as scratch is set, bridge seamlessly
            # into the real matmul stream to keep the HAM ramp alive (cold
            # cadence is ~2x warm; an idle gap triggers a fixed ~3.4us
            # half-speed window).
            psw = pspool.tile([JS, C], mybir.dt.float32, tag="psw", name="psw")
            for _ in range(N_WARMUP):
                nc.tensor.matmul(psw[:], scratch[:, 0:JS], scratch[:],
                                 start=True, stop=True)

            def filler():
                # cheap 100-col matmul: keeps the PE busy across a DMA-gate
                # stall for ~85ns warm instead of idling (which would cost a
                # 3.4us HAM re-throttle window)
                nc.tensor.matmul(psw[:, 0:JS], scratch[:, 0:JS],
                                 scratch[:, 0:JS], start=True, stop=True)

            def w_slice(jt, k):
                g, k0 = wmap[k]
                c0 = (k - k0) * JL + jt * JS
                return g[:, c0:c0 + JS]

            def x_slice(k):
                g, k0 = xmap[k]
                return g[:, (k - k0) * C:(k - k0 + 1) * C]

            # replicate [JS,12] coefficient vectors to [JS,192] in one 4D-AP
            # copy (coef lands early on the gpsimd queue) so the combines run
            # on flat APs.
            crep = pool.tile([JS, 3 * NJ * CA], bf16, tag="crep")
            src = coef_all[:].rearrange("p (g l) -> p g l", g=3 * NJ)
            dst = crep[:].rearrange("p (g l b) -> p g l b", g=3 * NJ, l=NL)
            nc.vector.tensor_copy(dst, src.broadcast_to([JS, 3 * NJ, NL, BL]))

            ps = [pspool.tile([JS, C], mybir.dt.float32, tag=f"ps{jt}",
                              name=f"ps{jt}")
                  for jt in range(NJ)]
            mm_order = [(jt, k) for k in range(6) for jt in range(NJ)]
            mm_order += [(jt, k) for jt in range(NJ) for k in range(6, NK)]
            for i, (jt, k) in enumerate(mm_order):
                if (jt, k) == (0, 2):
                    for _ in range(4):
                        filler()
                if (jt, k) == (0, 6):
                    for _ in range(2):
                        filler()
                nc.tensor.matmul(
                    ps[jt][:], w_slice(jt, k), x_slice(k),
                    start=(k == 0), stop=(k == NK - 1),
                )

            # epilogue: per-jt tanh on ACT, merged 3-jt combines on DVE,
            # one flat output DMA.
            t_all = pool.tile([JS, NJ * C], bf16, tag="t")
            res = pool.tile([JS, NJ * CA], bf16, tag="res")
            tmp = pool.tile([JS, NJ * CA], bf16, tag="tmp")
            tmp2 = pool.tile([JS, NJ * CA], bf16, tag="tmp2")
            for jt in range(NJ):
                nc.scalar.activation(
                    out=t_all[:, jt * C:(jt + 1) * C], in_=ps[jt][:],
                    func=mybir.ActivationFunctionType.Tanh,
                )
            t3 = t_all[:].rearrange("p (j c) -> p j c", j=NJ)

            def cre(i):
                return crep[:, i * NJ * CA:(i + 1) * NJ * CA].rearrange(
                    "p (j c) -> p j c", j=NJ)

            res3 = res[:].rearrange("p (j c) -> p j c", j=NJ)
            tmp3 = tmp[:].rearrange("p (j c) -> p j c", j=NJ)
            tmp23 = tmp2[:].rearrange("p (j c) -> p j c", j=NJ)
            ve = nc.vector
            ve.tensor_mul(res3, cre(0), t3[:, :, 0:CA])
            ve.tensor_mul(tmp3, cre(1), t3[:, :, CA + BL:CA + CU])
            ve.tensor_mul(tmp23, cre(2), t3[:, :, CA:CA + CA])
            ve.tensor_add(res[:], res[:], tmp[:])
            ve.tensor_add(res[:], res[:], tmp2[:])
            nc.sync.dma_start(out=out[:, :], in_=res[:])

    return nc


def _get_nc():
    global _cached
    if _cached is None:
        _cached = _build_nc()
        _cached.finalize()   # Bacc: runs reg alloc + codegen passes
    return _cached


def _host_coefs(alpha, fract, lambd, l):
    # All [12,...] fp32; compute in float64, cast at the end.
    a = alpha[:, 0].astype(np.float64)          # [12]
    f = fract[:, 0].astype(np.float64)          # [12]
    lam = lambd[:, 0, :, 0].astype(np.float64)  # [12, 200]
    ll = l[:, 0, :, 0].astype(np.float64)       # [12, 200]

    belta = np.zeros(NL)
    for la in range(NL):
        g_a1 = _gamma(a[la] + 1.0)
        belta[la] = sum(
            g_a1 / (_gamma(kk + 1.0) * _gamma(a[la] - kk + 1.0)) for kk in range(4)
        )
    cN = np.array([_gamma(a[la] + 1.0) / (6.0 * _gamma(a[la] - 2.0))
                   for la in range(NL)])

    # tile lambda/l from 200 -> 1200 (index n % 200)
    lam_t = np.tile(lam, (1, 6))                # [12, 1200]
    ll_t = np.tile(ll, (1, 6))                  # [12, 1200]

    inv_hf = (1.0 / H) ** f                     # 3**fract
    P = 2.0 * lam_t / belta[:, None] * inv_hf[:, None]
    Q = lam_t * ll_t / belta[:, None] / H
    R = Q * cN[:, None]
    return P, Q, R


def kernel(A, WW, train_init, alpha, fract, lambd, l, A_y_list):
    import ml_dtypes
    from concourse.bass_utils import run_bass_kernel_spmd

    bf16 = ml_dtypes.bfloat16

    A = np.asarray(A, dtype=np.float32)
    WW = np.asarray(WW, dtype=np.float32)
    train_init = np.asarray(train_init, dtype=np.float32)

    P, Q, R = _host_coefs(
        np.asarray(alpha, np.float32), np.asarray(fract, np.float32),
        np.asarray(lambd, np.float32), np.asarray(l, np.float32))

    Wneg = (-WW[:, :, 0]).astype(bf16)          # [1200, 1200]

    xts, wcs, coefs = {}, {}, {}
    for beta in range(PB):
        bsl = slice(beta * BL, (beta + 1) * BL)
        xa = A[:, bsl, :, 0].astype(bf16).transpose(2, 0, 1).reshape(N, CA)
        xu = train_init[bsl, :, :, 1].astype(bf16).transpose(1, 2, 0).reshape(
            N, CU)
        XT = np.concatenate([xa, xu], axis=1)                   # [1200, 400]
        # partition-major: [KT, NK*C], col = k*C + c
        xts[beta] = np.ascontiguousarray(
            XT.reshape(NK, KT, C).transpose(1, 0, 2).reshape(KT, NK * C))
    for g in range(PJ):
        gsl = slice(g * JL, (g + 1) * JL)
        # partition-major, k-major: col = k*JL + j
        wcs[g] = np.ascontiguousarray(
            Wneg[:, gsl].reshape(NK, KT, JL).transpose(1, 0, 2).reshape(
                KT, NK * JL))
        # coef [JS, 108]: col = kind*36 + jt*12 + la
        kinds = [M[:, gsl].reshape(NL, NJ, JS).transpose(2, 1, 0)
                 for M in (P, Q, R)]                            # [100, 3, 12]
        coefs[g] = np.ascontiguousarray(
            np.stack(kinds, axis=1).reshape(JS, 3 * NJ * NL).astype(bf16))

    in_maps = []
    for core in range(PB * PJ):
        beta, g = divmod(core, PJ)
        in_maps.append({"xt": xts[beta], "wc": wcs[g], "coef": coefs[g]})

    nc = _get_nc()
    res = run_bass_kernel_spmd(nc, in_maps, core_ids=list(range(PB * PJ)))
    kernel.last_results = res

    full = np.empty((B, NL, N), dtype=np.float32)
    for core in range(PB * PJ):
        beta, g = divmod(core, PJ)
        o = res.results[core]["out"]            # [JS, NJ*CA] bf16
        o = np.asarray(o).astype(np.float32).reshape(JS, NJ, NL, BL)
        # out[p, jt, la, b] -> full[b, la, g*JL + jt*JS + p]
        full[beta * BL:(beta + 1) * BL, :, g * JL:(g + 1) * JL] = (
            o.transpose(3, 2, 1, 0).reshape(BL, NL, JL))
    return full.reshape(B, NL, N, 1)
